# revision 19
# baseline (speedup 1.0000x reference)
"""MultiEmbedding (embedding_lookup) Trainium2 kernel.

Math: y[n, :] = sum_l weight[l, x[n, l], :]  for x1/x2 [65536, 8], weight [8, 1024, 1024].

Strategy (8 NeuronCores, data-parallel over tokens):
  - Concat x1+x2 -> 131072 tokens, 16384 per core.
  - Host casts the table to fp16 [8192, 1024] (row = l*1024 + k), halving
    gather traffic vs f32; packs flat indices l-major within each 128-token
    chunk (i = l*128 + t) into the wrapped int16 dma_gather layout.
  - Device, per 128-token chunk: SWDGE dma_gather pulls 1024 fp16 rows
    (2 KiB) HBM->SBUF, G[p, l, :] = row for (level l, token p); the TENSOR
    engine then runs 8 accumulating matmuls with an identity stationary:
    psum[t, d] += G[t-row of level l, d], i.e. the level-sum happens in the
    PE/PSUM in f32 (exact sum of fp16 values), landing token-major.
    One HWDGE DMA writes psum -> y[k*128:(k+1)*128, :] (4 KiB/partition).
  - DVE does nothing; no transposes; host just concatenates core outputs.
"""

import numpy as np

L, K, D = 8, 1024, 1024
T_TOTAL = 131072  # x1 + x2 tokens
N_CORES = 8
T_CORE = T_TOTAL // N_CORES  # 16384

# tunables
CHUNK_TOK = 256  # tokens per dma_gather (multiple of 128)
GBUFS = 3  # gather-tile buffering
PBUFS = 2  # psum buffering
N_QUEUES = 1  # SWDGE queues used round-robin
SINGLE_PACKET = False

_compiled = None


def _build(
    t_core=T_CORE,
    chunk_tok=CHUNK_TOK,
    gbufs=GBUFS,
    pbufs=PBUFS,
    n_queues=N_QUEUES,
    single_packet=SINGLE_PACKET,
    num_devices=N_CORES,
):
    import concourse.bass as bass  # noqa: F401
    import concourse.tile as tile
    from concourse import bacc, mybir

    # chunk schedule: taper the tail to 128-token chunks to shrink the
    # serial drain+compute after the last descriptor generation
    chunk_sizes = []
    rem = t_core
    while rem > 0:
        if rem > chunk_tok:
            chunk_sizes.append(chunk_tok)
            rem -= chunk_tok
        elif rem == chunk_tok and chunk_tok > 128:
            chunk_sizes += [128] * (chunk_tok // 128)
            rem = 0
        else:
            chunk_sizes.append(rem)
            rem = 0

    nc = bacc.Bacc(
        "TRN2",
        target_bir_lowering=False,
        debug=False,
        num_devices=num_devices,
        num_swdge_queues=n_queues,
    )
    w_ap = nc.dram_tensor(
        "w", [L * K, D], mybir.dt.float16, kind="ExternalInput"
    ).ap()
    idx_ap = nc.dram_tensor(
        "idx", [128, t_core * L // 16], mybir.dt.int16, kind="ExternalInput"
    ).ap()
    ident_ap = nc.dram_tensor(
        "ident", [128, 128], mybir.dt.float16, kind="ExternalInput"
    ).ap()
    y_ap = nc.dram_tensor(
        "y", [t_core, D], mybir.dt.float32, kind="ExternalOutput"
    ).ap()

    total_slots = t_core * L // 16
    slots0 = chunk_sizes[0] * L // 16  # chunk 0's idx slice, loaded first

    with tile.TileContext(nc) as tc:
        # split idx load: chunk 0's slice lands in ~1 us so descriptor
        # generation starts immediately; the bulk loads concurrently
        idx_a, free_idx_a = tc.tile([128, slots0], mybir.dt.int16, name="idx_a")
        idx_b, free_idx_b = tc.tile(
            [128, total_slots - slots0], mybir.dt.int16, name="idx_b"
        )
        ident_sb, free_ident = tc.tile([128, 128], mybir.dt.float16, name="ident_sb")
        nc.sync.dma_start(out=idx_a[:], in_=idx_ap[:, :slots0])
        nc.sync.dma_start(out=ident_sb[:], in_=ident_ap)
        nc.sync.dma_start(out=idx_b[:], in_=idx_ap[:, slots0:])

        with (
            tc.tile_pool(name="g", bufs=gbufs) as gpool,
            tc.tile_pool(name="yt", bufs=gbufs) as ypool,
            tc.psum_pool(name="ps", bufs=pbufs) as ppool,
        ):
            tok0 = 0
            slot0 = 0
            for k, csize in enumerate(chunk_sizes):
                num_idxs = csize * L
                blocks = csize // 128
                nslots = num_idxs // 16
                if k == 0:
                    idxs_ap = idx_a[:]
                else:
                    idxs_ap = idx_b[:, slot0 - slots0 : slot0 - slots0 + nslots]
                # G[p, b*L + l, :] = table row for (level l, token b*128+p)
                g = gpool.tile([128, blocks * L, D], mybir.dt.float16)
                nc.gpsimd.dma_gather(
                    out_ap=g[:],
                    in_ap=w_ap,
                    idxs_ap=idxs_ap,
                    num_idxs=num_idxs,
                    num_idxs_reg=num_idxs,
                    elem_size=D,
                    single_packet=single_packet,
                    queue_num=k % n_queues,
                )
                for b in range(blocks):
                    ps = ppool.tile([128, D], mybir.dt.float32)
                    # one matmul per (level, 512-wide half): PSUM banks are
                    # 512 f32, a single matmul may not cross a bank boundary
                    for h in range(2):
                        hs = slice(h * 512, (h + 1) * 512)
                        for l in range(L):
                            nc.tensor.matmul(
                                out=ps[:, hs],
                                lhsT=ident_sb[:],
                                rhs=g[:, b * L + l, hs],
                                start=(l == 0),
                                stop=(l == L - 1),
                            )
                    yt = ypool.tile([128, D], mybir.dt.float32)
                    nc.scalar.copy(out=yt[:], in_=ps[:])
                    t0 = tok0 + b * 128
                    nc.sync.dma_start(out=y_ap[t0 : t0 + 128], in_=yt[:])
                tok0 += csize
                slot0 += nslots
        free_ident()
        free_idx_b()
        free_idx_a()
    nc.compile()
    return nc


def _get_compiled():
    global _compiled
    if _compiled is None:
        _compiled = _build()
    return _compiled


def _pack_indices(x_core: np.ndarray) -> np.ndarray:
    """x_core [T, 8] int -> [128, T*8//16] int16 wrapped dma_gather layout.

    Within each 128-token chunk, flat order i = l*128 + t (l-major);
    value = l*1024 + x[t, l]. Wrapped: tile[p, s] = q[s*16 + p%16],
    replicated over 8 groups of 16 partitions.
    """
    t = x_core.shape[0]
    flat = (
        x_core.astype(np.int32) + (np.arange(L, dtype=np.int32) * K)[None, :]
    ).astype(np.int16)
    q = flat.reshape(t // 128, 128, L).transpose(0, 2, 1).reshape(-1)
    qr = q.reshape(-1, 16).T  # [16, S]
    return np.tile(qr, (8, 1)).copy()


def _prepare_in_maps(x1: np.ndarray, x2: np.ndarray, weight: np.ndarray):
    x = np.concatenate([np.asarray(x1), np.asarray(x2)], axis=0)
    w_f16 = np.ascontiguousarray(
        np.asarray(weight, dtype=np.float32).reshape(L * K, D).astype(np.float16)
    )
    ident = np.eye(128, dtype=np.float16)

    in_maps = []
    for c in range(N_CORES):
        xc = x[c * T_CORE : (c + 1) * T_CORE]
        in_maps.append({"w": w_f16, "idx": _pack_indices(xc), "ident": ident})
    return in_maps


def kernel(x1: np.ndarray, x2: np.ndarray, weight: np.ndarray):
    from concourse.bass_utils import run_bass_kernel_spmd

    nc = _get_compiled()
    in_maps = _prepare_in_maps(x1, x2, weight)

    res = _run_with_retry(run_bass_kernel_spmd, nc, in_maps)
    y_full = np.concatenate(
        [res.results[c]["y"] for c in range(N_CORES)], axis=0
    )
    return (y_full[: T_TOTAL // 2], y_full[T_TOTAL // 2 :])


def _retry_call(fn, attempts=6, sleep_s=75):
    """The axon-tunnelled device occasionally reports unrecoverable for a few
    minutes after a previous session crashed; back off and retry."""
    import time

    last = None
    for i in range(attempts):
        try:
            return fn()
        except Exception as e:  # noqa: BLE001 - jax.errors.JaxRuntimeError etc.
            last = e
            if i == attempts - 1:
                break
            try:
                import jax

                jax.clear_caches()
                import jax.extend.backend

                jax.extend.backend.clear_backends()
            except Exception:
                pass
            time.sleep(sleep_s)
    raise last


def _run_with_retry(run_fn, nc, in_maps, attempts=6, sleep_s=75):
    return _retry_call(
        lambda: run_fn(nc, in_maps, core_ids=list(range(N_CORES))),
        attempts=attempts,
        sleep_s=sleep_s,
    )


# revision 20
# speedup vs baseline: 1.0238x; 1.0238x over previous
"""MultiEmbedding (embedding_lookup) Trainium2 kernel.

Math: y[n, :] = sum_l weight[l, x[n, l], :]  for x1/x2 [65536, 8], weight [8, 1024, 1024].

Strategy (8 NeuronCores, data-parallel over tokens):
  - Concat x1+x2 -> 131072 tokens, 16384 per core.
  - Host casts the table to fp16 [8192, 1024] (row = l*1024 + k), halving
    gather traffic vs f32; packs flat indices l-major within each 128-token
    chunk (i = l*128 + t) into the wrapped int16 dma_gather layout.
  - Device, per 128-token chunk: SWDGE dma_gather pulls 1024 fp16 rows
    (2 KiB) HBM->SBUF, G[p, l, :] = row for (level l, token p); the TENSOR
    engine then runs 8 accumulating matmuls with an identity stationary:
    psum[t, d] += G[t-row of level l, d], i.e. the level-sum happens in the
    PE/PSUM in f32 (exact sum of fp16 values), landing token-major.
    One HWDGE DMA writes psum -> y[k*128:(k+1)*128, :] (4 KiB/partition).
  - DVE does nothing; no transposes; host just concatenates core outputs.
"""

import numpy as np

L, K, D = 8, 1024, 1024
T_TOTAL = 131072  # x1 + x2 tokens
N_CORES = 8
T_CORE = T_TOTAL // N_CORES  # 16384

# tunables
CHUNK_TOK = 256  # tokens per dma_gather (multiple of 128)
GBUFS = 4  # gather-tile buffering
PBUFS = 2  # psum buffering
N_QUEUES = 1  # SWDGE queues used round-robin
SINGLE_PACKET = False

_compiled = None


def _build(
    t_core=T_CORE,
    chunk_tok=CHUNK_TOK,
    gbufs=GBUFS,
    pbufs=PBUFS,
    n_queues=N_QUEUES,
    single_packet=SINGLE_PACKET,
    num_devices=N_CORES,
):
    import concourse.bass as bass  # noqa: F401
    import concourse.tile as tile
    from concourse import bacc, mybir

    # chunk schedule: taper the tail to 128-token chunks to shrink the
    # serial drain+compute after the last descriptor generation
    chunk_sizes = []
    rem = t_core
    while rem > 0:
        if rem > chunk_tok:
            chunk_sizes.append(chunk_tok)
            rem -= chunk_tok
        elif rem == chunk_tok and chunk_tok > 128:
            chunk_sizes += [128] * (chunk_tok // 128)
            rem = 0
        else:
            chunk_sizes.append(rem)
            rem = 0

    nc = bacc.Bacc(
        "TRN2",
        target_bir_lowering=False,
        debug=False,
        num_devices=num_devices,
        num_swdge_queues=n_queues,
    )
    w_ap = nc.dram_tensor(
        "w", [L * K, D], mybir.dt.float16, kind="ExternalInput"
    ).ap()
    idx_ap = nc.dram_tensor(
        "idx", [128, t_core * L // 16], mybir.dt.int16, kind="ExternalInput"
    ).ap()
    ident_ap = nc.dram_tensor(
        "ident", [128, 128], mybir.dt.float16, kind="ExternalInput"
    ).ap()
    y_ap = nc.dram_tensor(
        "y", [t_core, D], mybir.dt.float32, kind="ExternalOutput"
    ).ap()

    total_slots = t_core * L // 16
    slots0 = chunk_sizes[0] * L // 16  # chunk 0's idx slice, loaded first

    with tile.TileContext(nc) as tc:
        # split idx load: chunk 0's slice lands in ~1 us so descriptor
        # generation starts immediately; the bulk loads concurrently
        idx_a, free_idx_a = tc.tile([128, slots0], mybir.dt.int16, name="idx_a")
        idx_b, free_idx_b = tc.tile(
            [128, total_slots - slots0], mybir.dt.int16, name="idx_b"
        )
        ident_sb, free_ident = tc.tile([128, 128], mybir.dt.float16, name="ident_sb")
        nc.sync.dma_start(out=idx_a[:], in_=idx_ap[:, :slots0])
        nc.sync.dma_start(out=ident_sb[:], in_=ident_ap)
        nc.sync.dma_start(out=idx_b[:], in_=idx_ap[:, slots0:])

        with (
            tc.tile_pool(name="g", bufs=gbufs) as gpool,
            tc.tile_pool(name="yt", bufs=gbufs) as ypool,
            tc.psum_pool(name="ps", bufs=pbufs) as ppool,
        ):
            tok0 = 0
            slot0 = 0
            for k, csize in enumerate(chunk_sizes):
                num_idxs = csize * L
                blocks = csize // 128
                nslots = num_idxs // 16
                if k == 0:
                    idxs_ap = idx_a[:]
                else:
                    idxs_ap = idx_b[:, slot0 - slots0 : slot0 - slots0 + nslots]
                # G[p, b*L + l, :] = table row for (level l, token b*128+p)
                g = gpool.tile([128, blocks * L, D], mybir.dt.float16)
                nc.gpsimd.dma_gather(
                    out_ap=g[:],
                    in_ap=w_ap,
                    idxs_ap=idxs_ap,
                    num_idxs=num_idxs,
                    num_idxs_reg=num_idxs,
                    elem_size=D,
                    single_packet=single_packet,
                    queue_num=k % n_queues,
                )
                for b in range(blocks):
                    ps = ppool.tile([128, D], mybir.dt.float32)
                    # one matmul per (level, 512-wide half): PSUM banks are
                    # 512 f32, a single matmul may not cross a bank boundary
                    for h in range(2):
                        hs = slice(h * 512, (h + 1) * 512)
                        for l in range(L):
                            nc.tensor.matmul(
                                out=ps[:, hs],
                                lhsT=ident_sb[:],
                                rhs=g[:, b * L + l, hs],
                                start=(l == 0),
                                stop=(l == L - 1),
                            )
                    yt = ypool.tile([128, D], mybir.dt.float32)
                    nc.scalar.copy(out=yt[:], in_=ps[:])
                    t0 = tok0 + b * 128
                    nc.sync.dma_start(out=y_ap[t0 : t0 + 128], in_=yt[:])
                tok0 += csize
                slot0 += nslots
        free_ident()
        free_idx_b()
        free_idx_a()
    nc.compile()
    return nc


def _get_compiled():
    global _compiled
    if _compiled is None:
        _compiled = _build()
    return _compiled


def _pack_indices(x_core: np.ndarray) -> np.ndarray:
    """x_core [T, 8] int -> [128, T*8//16] int16 wrapped dma_gather layout.

    Within each 128-token chunk, flat order i = l*128 + t (l-major);
    value = l*1024 + x[t, l]. Wrapped: tile[p, s] = q[s*16 + p%16],
    replicated over 8 groups of 16 partitions.
    """
    t = x_core.shape[0]
    flat = (
        x_core.astype(np.int32) + (np.arange(L, dtype=np.int32) * K)[None, :]
    ).astype(np.int16)
    q = flat.reshape(t // 128, 128, L).transpose(0, 2, 1).reshape(-1)
    qr = q.reshape(-1, 16).T  # [16, S]
    return np.tile(qr, (8, 1)).copy()


def _prepare_in_maps(x1: np.ndarray, x2: np.ndarray, weight: np.ndarray):
    x = np.concatenate([np.asarray(x1), np.asarray(x2)], axis=0)
    w_f16 = np.ascontiguousarray(
        np.asarray(weight, dtype=np.float32).reshape(L * K, D).astype(np.float16)
    )
    ident = np.eye(128, dtype=np.float16)

    in_maps = []
    for c in range(N_CORES):
        xc = x[c * T_CORE : (c + 1) * T_CORE]
        in_maps.append({"w": w_f16, "idx": _pack_indices(xc), "ident": ident})
    return in_maps


def kernel(x1: np.ndarray, x2: np.ndarray, weight: np.ndarray):
    from concourse.bass_utils import run_bass_kernel_spmd

    nc = _get_compiled()
    in_maps = _prepare_in_maps(x1, x2, weight)

    res = _run_with_retry(run_bass_kernel_spmd, nc, in_maps)
    y_full = np.concatenate(
        [res.results[c]["y"] for c in range(N_CORES)], axis=0
    )
    return (y_full[: T_TOTAL // 2], y_full[T_TOTAL // 2 :])


def _retry_call(fn, attempts=6, sleep_s=75):
    """The axon-tunnelled device occasionally reports unrecoverable for a few
    minutes after a previous session crashed; back off and retry."""
    import time

    last = None
    for i in range(attempts):
        try:
            return fn()
        except Exception as e:  # noqa: BLE001 - jax.errors.JaxRuntimeError etc.
            last = e
            if i == attempts - 1:
                break
            try:
                import jax

                jax.clear_caches()
                import jax.extend.backend

                jax.extend.backend.clear_backends()
            except Exception:
                pass
            time.sleep(sleep_s)
    raise last


def _run_with_retry(run_fn, nc, in_maps, attempts=6, sleep_s=75):
    return _retry_call(
        lambda: run_fn(nc, in_maps, core_ids=list(range(N_CORES))),
        attempts=attempts,
        sleep_s=sleep_s,
    )
